# revision 48
# baseline (speedup 1.0000x reference)
"""Trainium2 Bass kernel for nn_MultiHeadAttention_8684423872640.

Math: the reference collapses algebraically. With
  s[m]   = Wfc[0, m // 64] / sqrt(64)
  Abar   = (Wk * s[:,None]).T @ Wq / L          # [1024, 1024] weights-only
  u      = Wk.T @ (s * bq)                      # [1024]
  qv     = Wq.T @ (s * bk) / L                  # [1024]
  c0     = (s * bk) @ bq + bfc[0]
the output for batch b is
  xsum_b = sum_l x[b, l, :]                     # [1024]
  w_eff  = Abar @ xsum_b + u                    # [1024]
  c      = qv @ xsum_b + c0
  out[b, l, 0] = x[b, l, :] @ w_eff + c

Sharding: data-parallel over B — core c handles batch c.

v3 pipeline (per core):
  - x ships as fp8-e4m3 (4 MiB; e3m4 measured 8e-2 rel-err on HW — its
    denormal range covers 20% of N(0,1) — e4m3 keeps denormals to ~1%).
    Abar/qv stay bf16 (entries ~1e-7; fp8 underflows even scaled).
  - DMA: x tiles + Abar on the two HWDGE rings, balanced 3.0 MiB each,
    emitted under tc.high_priority so issues precede any compute in each
    engine's queue. SWDGE only carries the tiny qv/u/c0.
  - Row-sums are engine-bound with fp8 inputs (DVE tree runs 1x, not the
    bf16 2x mode), so they're split across THREE engines per tile:
    DVE pairwise-tree / ACT activation-accum / GpSimd pairwise-tree.
  - Folds accumulate in PSUM across all 8 p-tiles; two warmup-MM bursts
    (x0- and x6-gated) keep the PE HAM clock warm for folds and pass-2.
  - Pass-2: 4-way column-tiled matvec (tile_position=(0,32j)), mixed
    dtype (bf16 w_eff stationary x fp8 moving); 2 waves of 4 chunks;
    strided-partition epilogue + one out-DMA per wave.
"""

import os
import sys
import functools
import numpy as np

B, L, N = 8, 4096, 1024
D_K = 64
NCORES = 8
PT = N // 128   # 8 feature tiles
LCH = 512       # pass-2 moving chunk (PSUM bank limit)
QW = L // 4     # tail-tile DMA quarter

# row-sum slice widths per full tile (DVE / ACT / GPS)



_TRN_REPO = "/opt/trn_rl_repo"


def _ensure_path():
    if _TRN_REPO not in sys.path and os.path.isdir(_TRN_REPO):
        sys.path.insert(0, _TRN_REPO)


# pass-2 w_eff dtype: 'mixed' = bf16 stationary (x stays fp8 moving);
# 'fp8' = w cast to e4m3 scaled x128 (both operands fp8)
_W_MODE = os.environ.get("KERNEL_W_MODE", "mixed")


@functools.lru_cache(maxsize=2)
def _build(w_mode: str = _W_MODE, warm1: int = 8, warm2: int = 16):
    _ensure_path()
    import concourse.bass as bass
    import concourse.tile as tile
    from concourse import bacc, mybir

    f32 = mybir.dt.float32
    bf16 = mybir.dt.bfloat16
    f8 = mybir.dt.float8e4
    wdt = bf16
    # Abar/qv/u pre-scaled x2^20 on host so Abar fits fp8e4 (its entries
    # ~1e-7 underflow unscaled); the epilogue divides back out.
    wscale = float(2 ** 20)

    nc = bacc.Bacc(
        "TRN2",
        target_bir_lowering=False,
        debug=False,
        enable_asserts=False,
        num_devices=NCORES,
    )

    xT = nc.dram_tensor("xT", [N, L], f8, kind="ExternalInput").ap()
    atr = nc.dram_tensor("atr", [128, PT * N], f8, kind="ExternalInput").ap()
    qv8 = nc.dram_tensor("qv8", [128, PT], f8, kind="ExternalInput").ap()
    u8 = nc.dram_tensor("u8", [128, PT], f32, kind="ExternalInput").ap()
    c0 = nc.dram_tensor("c0", [1, 1], f32, kind="ExternalInput").ap()
    out_d = nc.dram_tensor("out", [1, L], f32, kind="ExternalOutput").ap()

    with tile.TileContext(nc) as tc:
        with (
            tc.tile_pool(name="xpool", bufs=PT) as xpool,
            tc.tile_pool(name="cpool", bufs=1) as cpool,
            tc.tile_pool(name="spool", bufs=4) as spool,
            tc.tile_pool(name="xsums", bufs=PT + 6) as xsums,
            tc.tile_pool(name="scrp", bufs=3) as scr_p,
            tc.tile_pool(name="gscrp", bufs=3) as gscr_p,
            tc.tile_pool(name="wps", bufs=2, space="PSUM") as wps,
            tc.tile_pool(name="cps", bufs=1, space="PSUM") as cps,
            tc.tile_pool(name="ops", bufs=2, space="PSUM") as ops,
            tc.tile_pool(name="wrm", bufs=1, space="PSUM") as wrm,
        ):
            x_pr = [xpool.tile([128, 2 * L], f8, tag="x", name=f"xp{k}")
                    for k in range(3)]
            x6_t = xpool.tile([128, L], f8, tag="x", name="x6")
            x7_t = xpool.tile([128, L], f8, tag="x", name="x7")
            x_sb = [x_pr[i // 2][:, (i % 2) * L:(i % 2 + 1) * L]
                    for i in range(6)] + [x6_t[:], x7_t[:]]
            at_sb = cpool.tile([128, PT * N], f8, tag="at")
            qv_sb = cpool.tile([128, PT], f8, tag="qv")
            u_sb = cpool.tile([128, PT], f32, tag="u")
            c0_sb = cpool.tile([1, 1], f32, tag="c0")

            # ---- DMA: few, large transfers; issues pinned to queue front.
            # Tile pairs ride one dma_start each (3-level AP) so every
            # consumer's semaphore wait is a first-use threshold.
            half = PT * N // 2
            H = L // 2
            def xpair(eng, k):
                eng.dma_start(
                    x_pr[k].rearrange("p (s l) -> p s l", s=2),
                    xT[256 * k:256 * (k + 1), :]
                    .rearrange("(s p) l -> p s l", s=2))

            with tc.high_priority():
                nc.gpsimd.dma_start(qv_sb[:], qv8[:])
                nc.gpsimd.dma_start(u_sb[:], u8[:])
                nc.gpsimd.dma_start(c0_sb[:], c0[:])
                # sync ring: x01, at (1 MiB fp8), x6    (2.5 MiB)
                xpair(nc.sync, 0)
                nc.sync.dma_start(at_sb[:], atr[:])
                nc.sync.dma_start(x6_t[:], xT[768:896, :])
                # scalar ring: x23, x45, x7a, x7b       (2.5 MiB)
                xpair(nc.scalar, 1)
                xpair(nc.scalar, 2)
                nc.scalar.dma_start(x7_t[:, 0:H], xT[896:, 0:H])
                nc.scalar.dma_start(x7_t[:, H:], xT[896:, H:])

            # ---- row-sum helpers ----
            # Engine rates (HW-measured, fp8 in): ACT activation-accum
            # 0.76 ns/elem; DVE tensor_tensor 1.16 ns/out; GpSimd t_t
            # ~2.5-3.7 ns/out. Split each tile: ACT eats [0:AW] raw, DVE
            # level-1 halves the rest, GPS level-2, DVE reduces; GPS does
            # the final combine+cast (fp32+fp32 -> bf16 xm in one op).
            AW = 2304           # ACT raw share of a full tile

            # NOTE (HW-verified): per-column start=True MMs with the group
            # left open across interleaved columns lose all but the last
            # start-write. Use closed per-MM groups + DVE accumulation.
            c_ps = cps.tile([1, 1], f32, tag="cps")
            w8_acc = spool.tile([128, PT], f32, tag="w8acc")

            def fold(pt, xm):
                wp = wps.tile([128, PT], f32, tag="wp", name=f"wp{pt}")
                for nt in range(PT):
                    nc.tensor.matmul(
                        wp[:, nt:nt + 1],
                        at_sb[:, pt * N + nt * 128: pt * N + (nt + 1) * 128],
                        xm, start=True, stop=True)
                nc.tensor.matmul(
                    c_ps[:], qv_sb[:, pt:pt + 1], xm,
                    start=(pt == 0), stop=(pt == PT - 1))
                if pt == 0:
                    nc.vector.tensor_copy(w8_acc[:], wp[:])
                else:
                    nc.vector.tensor_add(w8_acc[:], w8_acc[:], wp[:])

            def finish_tile(pt, parts):
                """xm = bf16(parts0 + parts1) on GPS, then fold."""
                xm = xsums.tile([128, 1], bf16, tag="xm", name=f"xm{pt}")
                nc.gpsimd.tensor_add(xm[:], parts[:, 0:1], parts[:, 1:2])
                fold(pt, xm[:])
                return xm

            def reduce_full(pt, scr, gscr):
                """Full tile: ACT raw [0:AW] -> p0; DVE L1 [AW:L] -> scr;
                GPS L2 -> gscr; DVE reduce -> p1; GPS combines."""
                x_ = x_sb[pt]
                parts = xsums.tile([128, 2], f32, tag="pp", name=f"pp{pt}")
                nc.scalar.activation(
                    act_scr[:, 0:AW], x_[:, 0:AW],
                    mybir.ActivationFunctionType.Copy,
                    bias=0.0, accum_out=parts[:, 0:1])
                h = (L - AW) // 2   # 1088
                nc.vector.tensor_add(
                    scr[:, 0:h], x_[:, AW:AW + h], x_[:, AW + h:L])
                nc.gpsimd.tensor_add(
                    gscr[:, 0:h // 2], scr[:, 0:h // 2], scr[:, h // 2:h])
                nc.vector.tensor_reduce(
                    parts[:, 1:2], gscr[:, 0:h // 2],
                    axis=mybir.AxisListType.X, op=mybir.AluOpType.add)
                return finish_tile(pt, parts)

            # ---- warmup burst 1: fire HAM as soon as x0 lands ----
            wscr = wrm.tile([1, LCH], f32, tag="warm")
            for i in range(warm1):
                nc.tensor.matmul(
                    wscr[:], x_sb[0][:, 0:1], x_sb[0][:, 0:LCH],
                    start=(i == 0), stop=(i == warm1 - 1))

            # ---- pass 1: row-sums + folds in arrival order ----
            act_scr = cpool.tile([128, AW], f8, tag="ascr")
            HL1 = (L - AW) // 2
            xm6 = None
            for k, pt in enumerate([0, 1, 2, 3, 4, 5, 6]):
                if pt == 6:
                    # warmup burst 2 ahead of the fold/pass-2 tail
                    for i in range(warm2):
                        nc.tensor.matmul(
                            wscr[:], x_sb[6][:, H:H + 1], x_sb[6][:, H:H + LCH],
                            start=(i == 0), stop=(i == warm2 - 1))
                scr = scr_p.tile([128, HL1], bf16, tag="scr", name=f"scr{pt}")
                gscr = gscr_p.tile([128, HL1 // 2], bf16, tag="gscr",
                                   name=f"gscr{pt}")
                xm6 = reduce_full(pt, scr, gscr)

            # tile 7, two halves: ACT raw-accumulates 7a; DVE chain 7b
            parts7 = xsums.tile([128, 2], f32, tag="pp", name="pp7")
            nc.scalar.activation(
                act_scr[:, 0:H], x_sb[7][:, 0:H],
                mybir.ActivationFunctionType.Copy,
                bias=0.0, accum_out=parts7[:, 0:1])
            scr7 = scr_p.tile([128, 1024], bf16, tag="scr", name="scr7")
            nc.vector.tensor_add(
                scr7[:, 0:1024], x_sb[7][:, H:H + 1024], x_sb[7][:, H + 1024:L])
            nc.vector.tensor_reduce(
                parts7[:, 1:2], scr7[:, 0:1024],
                axis=mybir.AxisListType.X, op=mybir.AluOpType.add)
            finish_tile(7, parts7)

            # ---- finalize w_eff / c ----
            w_sb = spool.tile([128, PT], wdt, tag="weff")
            nc.vector.tensor_add(w_sb[:], w8_acc[:], u_sb[:])
            c_sb = spool.tile([1, 1], f32, tag="csb")
            nc.vector.tensor_scalar(
                c_sb[:], c_ps[:], 1.0 / wscale, c0_sb[0:1, 0:1],
                mybir.AluOpType.mult, mybir.AluOpType.add)
            c_bc = spool.tile([128, 1], f32, tag="cbc")
            nc.gpsimd.partition_broadcast(c_bc[:], c_sb[0:1, 0:1])

            # ---- pass 2: 4-way column-tiled matvec, 2 waves ----
            out_sb = cpool.tile([128, 2 * LCH], f32, tag="osb")
            for wave in range(2):
                o_ps = ops.tile([128, LCH], f32, tag="ops", name=f"o{wave}")
                for nt in range(PT):
                    for j in range(4):
                        lc = wave * 4 + j
                        nc.tensor.matmul(
                            o_ps[32 * j:32 * j + 1, :],
                            w_sb[:, nt:nt + 1],
                            x_sb[nt][:, lc * LCH:(lc + 1) * LCH],
                            start=(nt == 0), stop=(nt == PT - 1),
                            tile_position=(0, 32 * j))
                nc.vector.tensor_scalar(
                    out_sb[:, wave * LCH:(wave + 1) * LCH],
                    o_ps[:, :],
                    1.0 / wscale, c_bc[:, 0:1],
                    mybir.AluOpType.mult, mybir.AluOpType.add)
            # one store for all 8 chunks: dram chunk c=wave*4+j from
            # sbuf row 32j, cols wave*512+k
            dst = out_d[0:1, :].rearrange("p (w j k) -> p j w k", w=2, j=4)
            src = out_sb[0:97:32, 0:2 * LCH].rearrange(
                "p (w k) -> p w k", w=2)
            nc.sync.dma_start(dst, src)

    nc.compile()
    return nc


def _prep_host(inputs, w_mode=_W_MODE):
    """Fold weights on host (f64 accumulate) and lay out per-core arrays."""
    import ml_dtypes
    wscale = float(2 ** 20)

    Wq = np.asarray(inputs["Wq"], np.float64)
    bq = np.asarray(inputs["bq"], np.float64)
    Wk = np.asarray(inputs["Wk"], np.float64)
    bk = np.asarray(inputs["bk"], np.float64)
    Wfc = np.asarray(inputs["Wfc"], np.float64)
    bfc = np.asarray(inputs["bfc"], np.float64)

    s = np.repeat(Wfc[0], D_K) / np.sqrt(D_K)
    A = (Wk * s[:, None]).T @ Wq / L          # [n, p] ; w_eff = A @ xsum + u
    u = Wk.T @ (s * bq)
    qv = Wq.T @ (s * bk) / L
    c0 = float((s * bk) @ bq + bfc[0])

    bf16 = ml_dtypes.bfloat16
    f8 = ml_dtypes.float8_e4m3

    at = np.ascontiguousarray(A.T) * wscale
    atr = np.ascontiguousarray(
        at.reshape(PT, 128, N).transpose(1, 0, 2).reshape(128, PT * N)
    ).astype(f8)
    qv8 = np.ascontiguousarray(
        (qv * wscale).reshape(PT, 128).T).astype(f8)
    u8 = np.ascontiguousarray((u * wscale).reshape(PT, 128).T).astype(np.float32)
    c0a = np.full((1, 1), c0, np.float32)

    x = np.asarray(inputs["x"])
    shared = {"atr": atr, "qv8": qv8, "u8": u8, "c0": c0a}
    in_maps = []
    for c in range(NCORES):
        m = dict(shared)
        m["xT"] = np.ascontiguousarray(x[c].T).astype(f8)
        in_maps.append(m)
    return in_maps


LAST_RESULTS = None


def kernel(**inputs) -> np.ndarray:
    global LAST_RESULTS
    _ensure_path()
    from concourse.bass_utils import run_bass_kernel_spmd

    nc = _build(_W_MODE)
    in_maps = _prep_host(inputs, _W_MODE)
    kw = {}
    if os.environ.get("KERNEL_TRACE"):
        kw["trace"] = True
    res = run_bass_kernel_spmd(nc, in_maps, list(range(NCORES)), **kw)
    LAST_RESULTS = res
    out = np.stack([res.results[c]["out"].reshape(L, 1) for c in range(NCORES)])
    return out.astype(np.float32)


if __name__ == "__main__":
    rng = np.random.default_rng(0)
    demo = {
        "x": rng.standard_normal((B, L, N), np.float32),
        "Wq": rng.standard_normal((N, N), np.float32) * 0.03,
        "bq": rng.standard_normal((N,), np.float32) * 0.03,
        "Wk": rng.standard_normal((N, N), np.float32) * 0.03,
        "bk": rng.standard_normal((N,), np.float32) * 0.03,
        "Wfc": rng.standard_normal((1, 16), np.float32) * 0.25,
        "bfc": rng.standard_normal((1,), np.float32) * 0.25,
    }
    o = kernel(**demo)
    print("out", o.shape, o.dtype, float(np.abs(o).max()))
